# revision 23
# baseline (speedup 1.0000x reference)
"""Trainium2 Bass kernel for LocalWindowAttention.

Reference semantics (per batch b):
    pad seq 4000 -> 4096, split into 32 windows of 128 tokens.
    qkv = x @ w_qkv.T + b_qkv ; per-window per-head softmax(q k^T / sqrt(64)) @ v
    out = o @ w_out.T + b_out ; drop padded tail.

Sharding: data-parallel over batch. Core b computes batch b fully.

Per-core layout strategy (everything chosen so matmul contraction = partition dim):
  - x is staged feature-major  xT[e, t]  (e on partitions, 8 chunks of 128).
  - Q computed feature-major (f on partitions); K likewise, kept in its
    eviction layout (f on partitions = head-pairs of d). Score matmuls are
    K=64 contractions at base partition 0/64, so head pairs run CONCURRENTLY
    in the two PE row-groups (row tiling, ~2x); pair (2i, 2i+1) drains to
    psum slots (i, i+4) = different PSUM banks.
  - V computed token-major (t on partitions) so AV works with V as stationary:
        O_u[d, tq] = sum_tk V[tk, d] E[tk, tq]   (col-tiled pairs, pos (0, 0/64))
  - softmax denominators: 16 accumulating one-hot matmuls into a corner of
    the (already-consumed) half-0 score psum; reciprocal + f16 cast on DVE;
    broadcast back to O shape via (16 x 128) selector matmuls into two
    1-bank psum tiles. The AV outputs are evicted to SBUF bf16 on the
    SCALAR engine right after each AV half, so the DVE o-normalize
    multiplies read the broadcast straight from PSUM -- no r-eviction on
    the critical path.  (1/sqrt(64) is folded into w_q on the host; exp is
    computed without max-subtraction which is exact for softmax and safe
    here: |scores| <= ~3.)
  - out projection consumes O feature-major chunks directly, pipelined TWO
    windows behind attention (with an early pop in window 1 so the pipeline
    prime-up never leaves the PE idle long enough to trip the HAM
    clock-gate into a mid-kernel re-throttle).
  - PSUM discipline (exactly 8 banks, no rotation aliasing): pool psS
    (2 bufs x 2 banks) holds score halves -- half-0 doubles as den scratch,
    half-1's banks are reused for the AV outputs (their lifetimes follow the
    true dependency chain); pool psF (2 bufs x 1 bank) holds the
    out-projection halves; pool psR (2 tags x 1 bank) holds the broadcast
    halves. The Q/K/V projection accumulators cycle through tags
    (s, s, r-pair, s) so nothing ever waits on an unrelated tile.
  - tail trim: only 4000 of 4096 tokens are real. The last chunk's Q and K
    matmuls stream 416 columns instead of 512 (K's padded key columns are
    memset to zero so padded keys still contribute exp(0)=1 with v=0, as the
    reference requires) and the last window's attention matmuls stream 32
    query columns instead of 128.
  - ~48 dummy matmuls on a zeroed tile run during the initial weight DMA so
    the PE HAM clock-gate is already un-throttled (2.4 GHz) when real matmuls
    start.
All matmuls use bf16/fp16 operands (1 cycle/row on TRN2; fp32 is 4x slower).
Accumulation is always fp32 in PSUM.
"""

import sys
import numpy as np

for _p in ("/opt/trn_rl_repo", "/root/.axon_site/_ro/trn_rl_repo"):
    if _p not in sys.path:
        sys.path.append(_p)

import ml_dtypes

P = 128          # partitions
E = 1024         # embed dim
H = 16           # heads
D = 64           # head dim
W = 128          # window
B = 8            # batch
S = 4000         # seq len
SP = 4096        # padded seq len
NW = SP // W     # 32 windows
CW = 4           # windows per chunk
CT = CW * W      # 512 tokens per chunk
EC = 8           # e-chunks of 128

BF16 = ml_dtypes.bfloat16
F16 = np.float16

WARMUP = True

# slot permutation within a half: concurrent score pairs (pos 2i, 2i+1) write
# psum slots (i, i+4) so the two concurrent drains hit different PSUM banks.
# e-slot of head h = (h//8)*8 + SLOT[h%8].
SLOT = [0, 4, 1, 5, 2, 6, 3, 7]          # pos -> slot
POS = [SLOT.index(s) for s in range(8)]  # slot -> pos

_cache = {}


def build_nc(n_chunks, s_out, has_bqk, has_bout):
    """Build + compile the single-core Bass program (same program for all cores)."""
    from concourse import bacc, tile, mybir

    dt = mybir.dt
    AF = mybir.ActivationFunctionType

    nc = bacc.Bacc(None, target_bir_lowering=False, debug=False)

    xt_d = nc.dram_tensor("xt", [n_chunks, P, EC, CT], dt.bfloat16, kind="ExternalInput")
    wqkv_d = nc.dram_tensor("wqkv", [P, EC, 3 * E], dt.bfloat16, kind="ExternalInput")
    wout_d = nc.dram_tensor("wout", [P, EC, E], dt.bfloat16, kind="ExternalInput")
    oh_d = nc.dram_tensor("onehot", [P, H, H], dt.bfloat16, kind="ExternalInput")
    sel_d = nc.dram_tensor("sel", [H, EC, P], dt.float16, kind="ExternalInput")
    out_d = nc.dram_tensor("out", [s_out, E], dt.float32, kind="ExternalOutput")
    if has_bqk:
        bqk_d = nc.dram_tensor("bqk", [P, 2, EC], dt.float32, kind="ExternalInput")
    if has_bout:
        cb_d = nc.dram_tensor("cb", [P, 2, 512], dt.float32, kind="ExternalInput")

    with tile.TileContext(nc) as tc:
        with (
            tc.tile_pool(name="const", bufs=1) as constp,
            tc.tile_pool(name="xp", bufs=2) as xp,
            tc.tile_pool(name="qkp", bufs=2) as qkp,
            tc.tile_pool(name="kp", bufs=2) as kp,
            tc.tile_pool(name="vp", bufs=2) as vp,
            tc.tile_pool(name="ep", bufs=2) as ep,
            tc.tile_pool(name="op", bufs=3) as opool,
            tc.tile_pool(name="oup", bufs=2) as oup,
            tc.tile_pool(name="rp", bufs=2) as rp,
            tc.tile_pool(name="fpl", bufs=3) as fpl,
            tc.tile_pool(name="psS", bufs=2, space="PSUM") as psS,
            tc.tile_pool(name="psF", bufs=2, space="PSUM") as psF,
            tc.tile_pool(name="psR", bufs=1, space="PSUM") as psR,
        ):
            ps_alloc_n = [0]

            def ps_alloc(i):
                """projection accumulators cycle (s, s, r-pair, s); returns
                per-half (128, 512) psum handles. Each alloc only ever waits
                on a long-finished evict."""
                ps_alloc_n[0] += 1
                n = ps_alloc_n[0]
                if i % 4 == 2:
                    return [psR.tile([P, 512], dt.float32, tag=f"r{hh}",
                                     name=f"pj{n}_{hh}")[:, :]
                            for hh in range(2)]
                t = psS.tile([P, 2, 512], dt.float32, tag="s", name=f"pj{n}")
                return [t[:, 0], t[:, 1]]

            # ---- PE warm-up: the HAM clock gate defaults to 1.2 GHz and only
            # un-throttles after ~3.4us of sustained matmul activity. Fill the
            # initial DMA wait with dummy matmuls so real work runs at 2.4 GHz.
            if WARMUP:
                wz = constp.tile([P, P], dt.bfloat16, name="warmz")
                nc.vector.memset(wz[:], 0.0)
                ps_w = psF.tile([P, P], dt.float32, tag="f", name="warm")
                for _ in range(32):
                    nc.tensor.matmul(ps_w[:], wz[:], wz[:], start=True, stop=True)

            # startup-critical DMAs first. wqQ / chunk-0 x live in per-ec
            # TILES (tile-granular dependency tracking) so chunk 0's first
            # ec-outer Q matmul unblocks after two small transfers instead
            # of the full 8.4MB of weights.
            wqQ = [constp.tile([P, E], dt.bfloat16, name=f"wqQ{ec}")
                   for ec in range(EC)]
            xt0 = [constp.tile([P, CT], dt.bfloat16, name=f"xt0_{ec}")
                   for ec in range(EC)]
            nc.sync.dma_start(xt0[0][:], xt_d[0][:, 0, :])
            nc.sync.dma_start(wqQ[0][:, 0:P], wqkv_d[:, 0, 0:P])
            nc.sync.dma_start(wqQ[0][:, P:E], wqkv_d[:, 0, P:E])
            for ec in range(1, EC):
                nc.sync.dma_start(wqQ[ec][:], wqkv_d[:, ec, 0:E])
                nc.sync.dma_start(xt0[ec][:], xt_d[0][:, ec, :])
            wq = constp.tile([P, EC, 2 * E], dt.bfloat16)  # K and V blocks
            for ec in range(EC):
                nc.sync.dma_start(wq[:, ec, 0:E], wqkv_d[:, ec, E:2 * E])
            for ec in range(EC):
                nc.sync.dma_start(wq[:, ec, E:2 * E], wqkv_d[:, ec, 2 * E:3 * E])
            oh = constp.tile([P, H, H], dt.bfloat16)
            nc.sync.dma_start(oh[:], oh_d[:])
            sel = constp.tile([H, EC, P], dt.float16)
            nc.sync.dma_start(sel[:], sel_d[:])
            wo = constp.tile([P, EC, E], dt.bfloat16)
            for ec in range(EC):
                nc.sync.dma_start(wo[:, ec, :], wout_d[:, ec, :])
            if has_bqk:
                bqk = constp.tile([P, 2, EC], dt.float32)
                nc.sync.dma_start(bqk[:], bqk_d[:])
            if has_bout:
                cb = constp.tile([P, 2, 512], dt.float32)
                nc.sync.dma_start(cb[:], cb_d[:])

            def stage_a1(wi, k_tiles, q_sb, tq):
                """scores -> exp (quartered ACTs so the D chain can start
                early). K=64 row-tiled head pairs run concurrently. e lives
                in four QUARTER tiles so downstream matmuls only wait on the
                exp quarter they actually read (deps are tile-granular)."""
                e_t = [ep.tile([P, 8, W], dt.bfloat16, tag=f"e{k}",
                               name=f"e{k}")
                       for k in range(2)]
                ps_halves = []
                for half in range(2):
                    ps_s = psS.tile([P, 8, W], dt.float32, tag="s",
                                    name=f"s{half}")
                    for pos in range(8):
                        h = half * 8 + pos
                        fg, hf, rg = h // 4, (h % 4) // 2, h % 2
                        pr = slice(rg * D, rg * D + D)
                        nc.tensor.matmul(
                            ps_s[:, SLOT[pos], :tq],
                            k_tiles[fg][pr, hf, wi * W:(wi + 1) * W],
                            q_sb[pr, h // 2, wi * W:wi * W + tq],
                            start=True,
                            stop=True,
                        )
                    nc.scalar.activation(
                        e_t[half][:, :, :tq], ps_s[:, :, :tq], AF.Exp,
                    )
                    ps_halves.append(ps_s)

                def eslot(idx):
                    return e_t[idx // 8][:, idx % 8, :tq]
                return eslot, ps_halves

            def stage_d16(eslot, ps_s0, tq):
                """denominators D16[slot, tq] via one-hot accumulation matmuls
                into a corner of the half-0 score psum (its exp is done by
                now); recip -> f16 cast on DVE (small, off the o-mul path).
                Slot order matches exp-quarter completion order, so no den
                matmul ever waits on a later quarter."""
                for h in range(H):
                    nc.tensor.matmul(
                        ps_s0[0:16, 0, :tq], oh[:, h, :], eslot(h),
                        start=(h == 0), stop=(h == H - 1),
                    )
                rd32 = rp.tile([H, W], dt.float32, tag="rd32")
                nc.vector.reciprocal_approx_fast(rd32[:, :tq], ps_s0[0:16, 0, :tq])
                rd16 = rp.tile([H, W], dt.float16, tag="rd16")
                nc.vector.tensor_copy(rd16[:, :tq], rd32[:, :tq])
                return rd16

            def stage_av(wi, eslot, ps_s1, v_sb, tq):
                """AV matmuls (col-tiled pairs) into the half-1 score psum
                banks (exp there is done), each half immediately evicted to
                SBUF bf16 on the SCALAR engine -- the DVE o-multiplies then
                read the broadcast result straight from PSUM, with no
                r-eviction on the critical path."""
                o_un = []
                for hh in range(2):
                    for h in range(hh * 8, hh * 8 + 8):
                        cc = h // 2 - hh * 4
                        po = (h % 2) * D
                        nc.tensor.matmul(
                            ps_s1[po:po + D, hh * 4 + cc, :tq],
                            v_sb[:, wi, h // 8, (h % 8) * D:(h % 8) * D + D],
                            eslot(hh * 8 + SLOT[h % 8]),
                            start=True,
                            stop=True,
                        )
                    sl = slice(hh * 4, hh * 4 + 4)
                    ou = oup.tile([P, 4, W], dt.bfloat16, tag=f"ou{hh}",
                                  name=f"ou{hh}")
                    nc.scalar.activation(ou[:, :, :tq], ps_s1[:, sl, :tq],
                                         AF.Copy)
                    o_un.append(ou)
                return o_un

            def stage_r(rd16, tq):
                """broadcast recip to O shape in two 1-bank psum tiles."""
                ps_rs = []
                for hh in range(2):
                    ps_r = psR.tile([P, 4, W], dt.float32, tag=f"r{hh}",
                                    name=f"ps_r{hh}")
                    for j in range(4):
                        cc = hh * 4 + j
                        nc.tensor.matmul(
                            ps_r[:, j, :tq], sel[:, cc, :], rd16[:, :tq],
                            start=True, stop=True,
                        )
                    ps_rs.append(ps_r)
                return ps_rs

            def stage_muls(o_un, ps_rs, tq):
                """normalize: SBUF o_un x PSUM broadcast -> bf16 o halves."""
                o_halves = []
                for hh in range(2):
                    o_h = opool.tile([P, 4, W], dt.bfloat16, tag=f"o{hh}",
                                     name=f"o{hh}")
                    nc.vector.tensor_mul(o_h[:, :, :tq], o_un[hh][:, :, :tq],
                                         ps_rs[hh][:, :, :tq])
                    o_halves.append(o_h)
                return o_halves

            def bmm_one(ps_fh, o_halves, fh, cc):
                nc.tensor.matmul(
                    ps_fh[:, :],
                    o_halves[cc // 4][:, cc % 4, :],
                    wo[:, cc, fh * 512:(fh + 1) * 512],
                    start=(cc == 0),
                    stop=(cc == EC - 1),
                )

            def stage_b_out_half(ps_fh, row0, rows, fh):
                """evict + DMA one 512-feature half of the out projection.
                Eviction on DVE: the SCALAR engine is saturated by exp +
                r-eviction during the attention phase."""
                f_sb = fpl.tile([P, 512], dt.float32, tag=f"f{fh}", name=f"f{fh}")
                if has_bout:
                    nc.vector.tensor_add(f_sb[:], ps_fh[:, :], cb[:, fh, :])
                else:
                    nc.vector.tensor_copy(f_sb[:], ps_fh[:, :])
                nc.sync.dma_start(
                    out_d[row0:row0 + rows, fh * 512:(fh + 1) * 512],
                    f_sb[:rows],
                )

            pends = []
            for c in range(n_chunks):
                tcv = min(s_out - c * CT, CT)  # valid tokens in this chunk
                if c == 0:
                    xt = None
                else:
                    xt = xp.tile([P, EC, CT], dt.bfloat16, tag="xt")
                    nc.sync.dma_start(xt[:], xt_d[c])

                def xt_ec(ec, sl=slice(None)):
                    return xt0[ec][:, sl] if c == 0 else xt[:, ec, sl]

                q_sb = qkp.tile([P, EC, CT], dt.bfloat16, tag="q")
                v_sb = vp.tile([P, CW, 2, 512], dt.bfloat16, tag="v")

                # ---- Q (feature-major): psum[f_tile, t] ----
                if c == 0:
                    # ec-outer so the first matmul only needs the first two
                    # small DMAs; uses all 4 psum accumulators live (fg3
                    # borrows the two 1-bank out-projection buffers).
                    ps_qs = [ps_alloc(i) for i in range(3)]
                    ps_qs.append([psF.tile([P, 512], dt.float32, tag="f",
                                           name=f"ps_q3{hh}")[:, :]
                                  for hh in range(2)])
                    for ec in range(EC):
                        for fg in range(4):
                            for half in range(2):
                                ft = fg * 2 + half
                                nc.tensor.matmul(
                                    ps_qs[fg][half][:, :tcv],
                                    wqQ[ec][:, ft * P:ft * P + P],
                                    xt_ec(ec, slice(0, tcv)),
                                    start=(ec == 0),
                                    stop=(ec == EC - 1),
                                )
                    for fg in range(4):
                        for half in range(2):
                            ft = fg * 2 + half
                            if has_bqk:
                                nc.scalar.activation(
                                    q_sb[:, ft, :tcv], ps_qs[fg][half][:, :tcv],
                                    AF.Identity, bias=bqk[:, 0, ft:ft + 1],
                                )
                            else:
                                nc.scalar.activation(
                                    q_sb[:, ft, :tcv], ps_qs[fg][half][:, :tcv],
                                    AF.Copy,
                                )
                else:
                    for fg in range(4):
                        ps = ps_alloc(fg)
                        for half in range(2):
                            ft = fg * 2 + half
                            for ec in range(EC):
                                nc.tensor.matmul(
                                    ps[half][:, :tcv],
                                    wqQ[ec][:, ft * P:ft * P + P],
                                    xt_ec(ec, slice(0, tcv)),
                                    start=(ec == 0),
                                    stop=(ec == EC - 1),
                                )
                        for half in range(2):
                            ft = fg * 2 + half
                            if has_bqk:
                                nc.scalar.activation(
                                    q_sb[:, ft, :tcv], ps[half][:, :tcv],
                                    AF.Identity, bias=bqk[:, 0, ft:ft + 1],
                                )
                            else:
                                nc.scalar.activation(
                                    q_sb[:, ft, :tcv], ps[half][:, :tcv],
                                    AF.Copy,
                                )

                # ---- K (feature-major). Evicted into k_tiles: head 2*ft+hh
                # lives at partitions 64hh.. of tile fg, half ft%2. Padded key
                # columns (last chunk) are memset to zero: the reference's
                # zero-padded x gives k=0 there, so padded keys contribute
                # exp(0)=1 with v=0. ----
                k_tiles = [kp.tile([P, 2, CT], dt.bfloat16, tag=f"k{fg}",
                                   name=f"k{fg}")
                           for fg in range(4)]
                for fg in range(4):
                    ps = ps_alloc(fg)
                    for half in range(2):
                        ft = fg * 2 + half
                        off = ft * P
                        for ec in range(EC):
                            nc.tensor.matmul(
                                ps[half][:, :tcv],
                                wq[:, ec, off:off + P],
                                xt_ec(ec, slice(0, tcv)),
                                start=(ec == 0),
                                stop=(ec == EC - 1),
                            )
                    if tcv < CT:
                        nc.gpsimd.memset(k_tiles[fg][:, :, tcv:], 0.0)
                    for half in range(2):
                        ft = fg * 2 + half
                        if has_bqk:
                            nc.scalar.activation(
                                k_tiles[fg][:, half, :tcv], ps[half][:, :tcv],
                                AF.Identity, bias=bqk[:, 1, ft:ft + 1],
                            )
                        else:
                            nc.scalar.activation(
                                k_tiles[fg][:, half, :tcv], ps[half][:, :tcv],
                                AF.Copy,
                            )

                # ---- V (token-major): psum[t, f] per window ----
                for wi in range(CW):
                    ps = ps_alloc(wi)
                    for fh in range(2):
                        off = E + fh * 512
                        for ec in range(EC):
                            nc.tensor.matmul(
                                ps[fh][:, :],
                                xt_ec(ec, slice(wi * W, (wi + 1) * W)),
                                wq[:, ec, off:off + 512],
                                start=(ec == 0),
                                stop=(ec == EC - 1),
                            )
                    for fh in range(2):
                        nc.vector.tensor_copy(v_sb[:, wi, fh], ps[fh][:, :])

                # ---- attention (A) + out-projection (B), software-pipelined:
                # B(w-1) is emitted inside A(w) so the PE has big streams to
                # hide the D-chain ldweights and the evict->normalize latency.
                # Window body; the out-projection runs with a lag of TWO
                # windows (bmm(w-2) inside window w) so its o stationaries
                # are always long-ready and the PE never waits on the
                # r-evict/o-multiply chain.
                for wi in range(CW):
                    g = c * CW + wi
                    row0 = g * W
                    rows = min(s_out - row0, W)
                    if rows <= 0:
                        continue
                    tq = rows
                    eslot, (ps_s0, ps_s1) = stage_a1(wi, k_tiles, q_sb, tq)
                    # early-pop in window 1: the lag-2 pipeline leaves the
                    # first windows without out-projection fill, and the bare
                    # dependency-chain idle can trip the HAM clock-gate into
                    # a mid-kernel re-throttle.
                    pend = pends.pop(0) if (len(pends) >= 2 or
                                            (g == 1 and pends)) else None
                    if pend is not None:
                        ps_f0 = psF.tile([P, 512], dt.float32, tag="f",
                                         name="ps_f0")
                        for cc in range(EC):
                            bmm_one(ps_f0, pend[0], 0, cc)
                    rd16 = stage_d16(eslot, ps_s0, tq)
                    o_un = stage_av(wi, eslot, ps_s1, v_sb, tq)
                    ps_rs = stage_r(rd16, tq)
                    o_halves = stage_muls(o_un, ps_rs, tq)
                    if pend is not None:
                        stage_b_out_half(ps_f0, pend[1], pend[2], 0)
                        ps_f1 = psF.tile([P, 512], dt.float32, tag="f",
                                         name="ps_f1")
                        for cc in range(EC):
                            bmm_one(ps_f1, pend[0], 1, cc)
                        stage_b_out_half(ps_f1, pend[1], pend[2], 1)
                    pends.append((o_halves, row0, rows))

            for pend in pends:
                ps_f0 = psF.tile([P, 512], dt.float32, tag="f", name="ps_f0")
                for cc in range(EC):
                    bmm_one(ps_f0, pend[0], 0, cc)
                stage_b_out_half(ps_f0, pend[1], pend[2], 0)
                ps_f1 = psF.tile([P, 512], dt.float32, tag="f", name="ps_f1")
                for cc in range(EC):
                    bmm_one(ps_f1, pend[0], 1, cc)
                stage_b_out_half(ps_f1, pend[1], pend[2], 1)

    nc.compile()
    return nc


def prep_inputs(x, w_qkv, b_qkv, w_out, b_out, n_chunks, s_out):
    """Host-side staging: pad, transpose, cast, fold scale into w_q."""
    sp = n_chunks * CT
    nb = x.shape[0]

    wqkvT = np.ascontiguousarray(w_qkv.T).astype(np.float32).copy()
    wqkvT[:, :E] *= 1.0 / np.sqrt(D)
    wqkv_sb = np.ascontiguousarray(
        wqkvT.reshape(EC, P, 3 * E).transpose(1, 0, 2)
    ).astype(BF16)

    woutT = np.ascontiguousarray(w_out.T)
    wout_sb = np.ascontiguousarray(
        woutT.reshape(EC, P, E).transpose(1, 0, 2)
    ).astype(BF16)

    oh = np.zeros((P, H, H), dtype=BF16)
    for h in range(H):
        oh[:, h, h] = 1

    # rd16 row r holds 1/den of e-slot r = head (r//8)*8 + POS[r%8];
    # output partition p of chunk cc wants head 2cc + p//64.
    selm = np.zeros((H, EC, P), dtype=F16)
    for cc in range(EC):
        for m in range(P):
            h = 2 * cc + m // D
            r = (h // 8) * 8 + SLOT[h % 8]
            selm[r, cc, m] = 1

    base = {"wqkv": wqkv_sb, "wout": wout_sb, "onehot": oh, "sel": selm}

    has_bqk = bool(np.any(b_qkv[:2 * E]))
    has_bout = bool(np.any(b_out)) or bool(np.any(b_qkv[2 * E:]))
    if has_bqk:
        bqk = np.stack(
            [b_qkv[:E].reshape(EC, P).T / np.sqrt(D),
             b_qkv[E:2 * E].reshape(EC, P).T], axis=1
        ).astype(np.float32)  # (P, 2, EC)
        base["bqk"] = np.ascontiguousarray(bqk)
    if has_bout:
        cbv = (b_out + b_qkv[2 * E:] @ w_out.T).astype(np.float32)  # (E,)
        base["cb"] = np.ascontiguousarray(
            np.broadcast_to(cbv.reshape(1, 2, 512), (P, 2, 512))
        ).copy()

    in_maps = []
    for b in range(nb):
        xp_ = np.zeros((sp, E), dtype=np.float32)
        xp_[:min(s_out, x.shape[1])] = x[b][:s_out]
        xT = np.ascontiguousarray(xp_.T)  # (E, sp)
        xt_sb = np.ascontiguousarray(
            xT.reshape(EC, P, n_chunks, CT).transpose(2, 1, 0, 3)
        ).astype(BF16)  # (n_chunks, P, EC, CT)
        m = dict(base)
        m["xt"] = xt_sb
        in_maps.append(m)
    return in_maps, has_bqk, has_bout


def run(x, w_qkv, b_qkv, w_out, b_out, n_chunks=NW // CW, s_out=S, trace=False):
    from concourse import bass_utils

    in_maps, has_bqk, has_bout = prep_inputs(
        x, w_qkv, b_qkv, w_out, b_out, n_chunks, s_out
    )
    key = (n_chunks, s_out, has_bqk, has_bout)
    if key not in _cache:
        _cache[key] = build_nc(*key)
    nc = _cache[key]

    res = bass_utils.run_bass_kernel_spmd(
        nc, in_maps, core_ids=list(range(len(in_maps))), trace=trace,
    )
    out = np.stack([r["out"] for r in res.results], axis=0)
    return out, res


def kernel(x, w_qkv, b_qkv, w_out, b_out):
    x = np.asarray(x, dtype=np.float32)
    w_qkv = np.asarray(w_qkv, dtype=np.float32)
    b_qkv = np.asarray(b_qkv, dtype=np.float32)
    w_out = np.asarray(w_out, dtype=np.float32)
    b_out = np.asarray(b_out, dtype=np.float32)
    out, _ = run(x, w_qkv, b_qkv, w_out, b_out)
    return out


# revision 24
# speedup vs baseline: 1.0006x; 1.0006x over previous
"""Trainium2 Bass kernel for LocalWindowAttention.

Reference semantics (per batch b):
    pad seq 4000 -> 4096, split into 32 windows of 128 tokens.
    qkv = x @ w_qkv.T + b_qkv ; per-window per-head softmax(q k^T / sqrt(64)) @ v
    out = o @ w_out.T + b_out ; drop padded tail.

Sharding: data-parallel over batch. Core b computes batch b fully.

Per-core layout strategy (everything chosen so matmul contraction = partition dim):
  - x is staged feature-major  xT[e, t]  (e on partitions, 8 chunks of 128).
  - Q computed feature-major (f on partitions); K likewise, kept in its
    eviction layout (f on partitions = head-pairs of d). Score matmuls are
    K=64 contractions at base partition 0/64, so head pairs run CONCURRENTLY
    in the two PE row-groups (row tiling, ~2x); pair (2i, 2i+1) drains to
    psum slots (i, i+4) = different PSUM banks.
  - V computed token-major (t on partitions) so AV works with V as stationary:
        O_u[d, tq] = sum_tk V[tk, d] E[tk, tq]   (col-tiled pairs, pos (0, 0/64))
  - softmax denominators: 16 accumulating one-hot matmuls into a corner of
    the (already-consumed) half-0 score psum; reciprocal + f16 cast on DVE;
    broadcast back to O shape via (16 x 128) selector matmuls into two
    1-bank psum tiles. The AV outputs are evicted to SBUF bf16 on the
    SCALAR engine right after each AV half, so the DVE o-normalize
    multiplies read the broadcast straight from PSUM -- no r-eviction on
    the critical path.  (1/sqrt(64) is folded into w_q on the host; exp is
    computed without max-subtraction which is exact for softmax and safe
    here: |scores| <= ~3.)
  - out projection consumes O feature-major chunks directly, pipelined TWO
    windows behind attention (with an early pop in window 1 so the pipeline
    prime-up never leaves the PE idle long enough to trip the HAM
    clock-gate into a mid-kernel re-throttle).
  - PSUM discipline (exactly 8 banks, no rotation aliasing): pool psS
    (2 bufs x 2 banks) holds score halves -- half-0 doubles as den scratch,
    half-1's banks are reused for the AV outputs (their lifetimes follow the
    true dependency chain); pool psF (2 bufs x 1 bank) holds the
    out-projection halves; pool psR (2 tags x 1 bank) holds the broadcast
    halves. The Q/K/V projection accumulators cycle through tags
    (s, s, r-pair, s) so nothing ever waits on an unrelated tile.
  - tail trim: only 4000 of 4096 tokens are real. The last chunk's Q and K
    matmuls stream 416 columns instead of 512 (K's padded key columns are
    memset to zero so padded keys still contribute exp(0)=1 with v=0, as the
    reference requires) and the last window's attention matmuls stream 32
    query columns instead of 128.
  - ~48 dummy matmuls on a zeroed tile run during the initial weight DMA so
    the PE HAM clock-gate is already un-throttled (2.4 GHz) when real matmuls
    start.
All matmuls use bf16/fp16 operands (1 cycle/row on TRN2; fp32 is 4x slower).
Accumulation is always fp32 in PSUM.
"""

import sys
import numpy as np

for _p in ("/opt/trn_rl_repo", "/root/.axon_site/_ro/trn_rl_repo"):
    if _p not in sys.path:
        sys.path.append(_p)

import ml_dtypes

P = 128          # partitions
E = 1024         # embed dim
H = 16           # heads
D = 64           # head dim
W = 128          # window
B = 8            # batch
S = 4000         # seq len
SP = 4096        # padded seq len
NW = SP // W     # 32 windows
CW = 4           # windows per chunk
CT = CW * W      # 512 tokens per chunk
EC = 8           # e-chunks of 128

BF16 = ml_dtypes.bfloat16
F16 = np.float16

WARMUP = True

# slot permutation within a half: concurrent score pairs (pos 2i, 2i+1) write
# psum slots (i, i+4) so the two concurrent drains hit different PSUM banks.
# e-slot of head h = (h//8)*8 + SLOT[h%8].
SLOT = [0, 4, 1, 5, 2, 6, 3, 7]          # pos -> slot
POS = [SLOT.index(s) for s in range(8)]  # slot -> pos

_cache = {}


def build_nc(n_chunks, s_out, has_bqk, has_bout):
    """Build + compile the single-core Bass program (same program for all cores)."""
    from concourse import bacc, tile, mybir

    dt = mybir.dt
    AF = mybir.ActivationFunctionType

    nc = bacc.Bacc(None, target_bir_lowering=False, debug=False)

    xt_d = nc.dram_tensor("xt", [n_chunks, P, EC, CT], dt.bfloat16, kind="ExternalInput")
    wqkv_d = nc.dram_tensor("wqkv", [P, EC, 3 * E], dt.bfloat16, kind="ExternalInput")
    wout_d = nc.dram_tensor("wout", [P, EC, E], dt.bfloat16, kind="ExternalInput")
    oh_d = nc.dram_tensor("onehot", [P, H, H], dt.bfloat16, kind="ExternalInput")
    sel_d = nc.dram_tensor("sel", [H, EC, P], dt.float16, kind="ExternalInput")
    out_d = nc.dram_tensor("out", [s_out, E], dt.float32, kind="ExternalOutput")
    if has_bqk:
        bqk_d = nc.dram_tensor("bqk", [P, 2, EC], dt.float32, kind="ExternalInput")
    if has_bout:
        cb_d = nc.dram_tensor("cb", [P, 2, 512], dt.float32, kind="ExternalInput")

    with tile.TileContext(nc) as tc:
        with (
            tc.tile_pool(name="const", bufs=1) as constp,
            tc.tile_pool(name="xp", bufs=2) as xp,
            tc.tile_pool(name="qkp", bufs=2) as qkp,
            tc.tile_pool(name="kp", bufs=2) as kp,
            tc.tile_pool(name="vp", bufs=2) as vp,
            tc.tile_pool(name="ep", bufs=2) as ep,
            tc.tile_pool(name="op", bufs=3) as opool,
            tc.tile_pool(name="oup", bufs=2) as oup,
            tc.tile_pool(name="rp", bufs=2) as rp,
            tc.tile_pool(name="fpl", bufs=3) as fpl,
            tc.tile_pool(name="psS", bufs=2, space="PSUM") as psS,
            tc.tile_pool(name="psF", bufs=2, space="PSUM") as psF,
            tc.tile_pool(name="psR", bufs=1, space="PSUM") as psR,
        ):
            ps_alloc_n = [0]

            def ps_alloc(i):
                """projection accumulators cycle (s, s, r-pair, s); returns
                per-half (128, 512) psum handles. Each alloc only ever waits
                on a long-finished evict."""
                ps_alloc_n[0] += 1
                n = ps_alloc_n[0]
                if i % 4 == 2:
                    return [psR.tile([P, 512], dt.float32, tag=f"r{hh}",
                                     name=f"pj{n}_{hh}")[:, :]
                            for hh in range(2)]
                t = psS.tile([P, 2, 512], dt.float32, tag="s", name=f"pj{n}")
                return [t[:, 0], t[:, 1]]

            # ---- PE warm-up: the HAM clock gate defaults to 1.2 GHz and only
            # un-throttles after ~3.4us of sustained matmul activity. Fill the
            # initial DMA wait with dummy matmuls so real work runs at 2.4 GHz.
            if WARMUP:
                wz = constp.tile([P, P], dt.bfloat16, name="warmz")
                nc.vector.memset(wz[:], 0.0)
                ps_w = psF.tile([P, P], dt.float32, tag="f", name="warm")
                for _ in range(32):
                    nc.tensor.matmul(ps_w[:], wz[:], wz[:], start=True, stop=True)

            # startup-critical DMAs first. wqQ / chunk-0 x live in per-ec
            # TILES (tile-granular dependency tracking) so chunk 0's first
            # ec-outer Q matmul unblocks after two small transfers instead
            # of the full 8.4MB of weights.
            wqQ = [constp.tile([P, E], dt.bfloat16, name=f"wqQ{ec}")
                   for ec in range(EC)]
            xt0 = [constp.tile([P, CT], dt.bfloat16, name=f"xt0_{ec}")
                   for ec in range(EC)]
            nc.sync.dma_start(xt0[0][:], xt_d[0][:, 0, :])
            nc.sync.dma_start(wqQ[0][:, 0:P], wqkv_d[:, 0, 0:P])
            nc.sync.dma_start(wqQ[0][:, P:E], wqkv_d[:, 0, P:E])
            for ec in range(1, EC):
                nc.sync.dma_start(wqQ[ec][:], wqkv_d[:, ec, 0:E])
                nc.sync.dma_start(xt0[ec][:], xt_d[0][:, ec, :])
            wq = constp.tile([P, EC, 2 * E], dt.bfloat16)  # K and V blocks
            for ec in range(EC):
                nc.sync.dma_start(wq[:, ec, 0:E], wqkv_d[:, ec, E:2 * E])
            for ec in range(EC):
                nc.sync.dma_start(wq[:, ec, E:2 * E], wqkv_d[:, ec, 2 * E:3 * E])
            oh = constp.tile([P, H, H], dt.bfloat16)
            nc.sync.dma_start(oh[:], oh_d[:])
            sel = constp.tile([H, EC, P], dt.float16)
            nc.sync.dma_start(sel[:], sel_d[:])
            wo = constp.tile([P, EC, E], dt.bfloat16)
            for ec in range(EC):
                nc.sync.dma_start(wo[:, ec, :], wout_d[:, ec, :])
            if has_bqk:
                bqk = constp.tile([P, 2, EC], dt.float32)
                nc.sync.dma_start(bqk[:], bqk_d[:])
            if has_bout:
                cb = constp.tile([P, 2, 512], dt.float32)
                nc.sync.dma_start(cb[:], cb_d[:])

            def stage_a1(wi, k_tiles, q_sb, tq):
                """scores -> exp (quartered ACTs so the D chain can start
                early). K=64 row-tiled head pairs run concurrently. e lives
                in four QUARTER tiles so downstream matmuls only wait on the
                exp quarter they actually read (deps are tile-granular)."""
                e_t = [ep.tile([P, 8, W], dt.bfloat16, tag=f"e{k}",
                               name=f"e{k}")
                       for k in range(2)]
                ps_halves = []
                for half in range(2):
                    ps_s = psS.tile([P, 8, W], dt.float32, tag="s",
                                    name=f"s{half}")
                    for pos in range(8):
                        h = half * 8 + pos
                        fg, hf, rg = h // 4, (h % 4) // 2, h % 2
                        pr = slice(rg * D, rg * D + D)
                        nc.tensor.matmul(
                            ps_s[:, SLOT[pos], :tq],
                            k_tiles[fg][pr, hf, wi * W:(wi + 1) * W],
                            q_sb[pr, h // 2, wi * W:wi * W + tq],
                            start=True,
                            stop=True,
                        )
                    nc.scalar.activation(
                        e_t[half][:, :, :tq], ps_s[:, :, :tq], AF.Exp,
                    )
                    ps_halves.append(ps_s)

                def eslot(idx):
                    return e_t[idx // 8][:, idx % 8, :tq]
                return eslot, ps_halves

            def stage_d16(eslot, ps_s0, tq):
                """denominators D16[slot, tq] via one-hot accumulation matmuls
                into a corner of the half-0 score psum (its exp is done by
                now); recip -> f16 cast on DVE (small, off the o-mul path).
                Slot order matches exp-quarter completion order, so no den
                matmul ever waits on a later quarter."""
                for h in range(H):
                    nc.tensor.matmul(
                        ps_s0[0:16, 0, :tq], oh[:, h, :], eslot(h),
                        start=(h == 0), stop=(h == H - 1),
                    )
                rd32 = rp.tile([H, W], dt.float32, tag="rd32")
                nc.vector.reciprocal_approx_fast(rd32[:, :tq], ps_s0[0:16, 0, :tq])
                rd16 = rp.tile([H, W], dt.float16, tag="rd16")
                nc.vector.tensor_copy(rd16[:, :tq], rd32[:, :tq])
                return rd16

            def stage_av(wi, eslot, ps_s1, v_sb, tq):
                """AV matmuls (col-tiled pairs) into the half-1 score psum
                banks (exp there is done), each half immediately evicted to
                SBUF bf16 on the SCALAR engine -- the DVE o-multiplies then
                read the broadcast result straight from PSUM, with no
                r-eviction on the critical path."""
                o_un = []
                for hh in range(2):
                    for h in range(hh * 8, hh * 8 + 8):
                        cc = h // 2 - hh * 4
                        po = (h % 2) * D
                        nc.tensor.matmul(
                            ps_s1[po:po + D, hh * 4 + cc, :tq],
                            v_sb[:, wi, h // 8, (h % 8) * D:(h % 8) * D + D],
                            eslot(hh * 8 + SLOT[h % 8]),
                            start=True,
                            stop=True,
                        )
                    sl = slice(hh * 4, hh * 4 + 4)
                    ou = oup.tile([P, 4, W], dt.bfloat16, tag=f"ou{hh}",
                                  name=f"ou{hh}")
                    nc.scalar.activation(ou[:, :, :tq], ps_s1[:, sl, :tq],
                                         AF.Copy)
                    o_un.append(ou)
                return o_un

            def stage_r(rd16, tq):
                """broadcast recip to O shape in two 1-bank psum tiles."""
                ps_rs = []
                for hh in range(2):
                    ps_r = psR.tile([P, 4, W], dt.float32, tag=f"r{hh}",
                                    name=f"ps_r{hh}")
                    for j in range(4):
                        cc = hh * 4 + j
                        nc.tensor.matmul(
                            ps_r[:, j, :tq], sel[:, cc, :], rd16[:, :tq],
                            start=True, stop=True,
                        )
                    ps_rs.append(ps_r)
                return ps_rs

            def stage_muls(o_un, ps_rs, tq):
                """normalize: SBUF o_un x PSUM broadcast -> bf16 o halves."""
                o_halves = []
                for hh in range(2):
                    o_h = opool.tile([P, 4, W], dt.bfloat16, tag=f"o{hh}",
                                     name=f"o{hh}")
                    nc.vector.tensor_mul(o_h[:, :, :tq], o_un[hh][:, :, :tq],
                                         ps_rs[hh][:, :, :tq])
                    o_halves.append(o_h)
                return o_halves

            def bmm_one(ps_fh, o_halves, fh, cc):
                nc.tensor.matmul(
                    ps_fh[:, :],
                    o_halves[cc // 4][:, cc % 4, :],
                    wo[:, cc, fh * 512:(fh + 1) * 512],
                    start=(cc == 0),
                    stop=(cc == EC - 1),
                )

            def stage_b_out_half(ps_fh, row0, rows, fh):
                """evict + DMA one 512-feature half of the out projection.
                Eviction on DVE: the SCALAR engine is saturated by exp +
                r-eviction during the attention phase."""
                f_sb = fpl.tile([P, 512], dt.float32, tag=f"f{fh}", name=f"f{fh}")
                if has_bout:
                    nc.vector.tensor_add(f_sb[:], ps_fh[:, :], cb[:, fh, :])
                else:
                    nc.vector.tensor_copy(f_sb[:], ps_fh[:, :])
                nc.sync.dma_start(
                    out_d[row0:row0 + rows, fh * 512:(fh + 1) * 512],
                    f_sb[:rows],
                )

            pends = []
            for c in range(n_chunks):
                tcv = min(s_out - c * CT, CT)  # valid tokens in this chunk
                if c == 0:
                    xt = None
                else:
                    xt = xp.tile([P, EC, CT], dt.bfloat16, tag="xt")
                    nc.sync.dma_start(xt[:], xt_d[c])

                def xt_ec(ec, sl=slice(None)):
                    return xt0[ec][:, sl] if c == 0 else xt[:, ec, sl]

                q_sb = qkp.tile([P, EC, CT], dt.bfloat16, tag="q")
                v_sb = vp.tile([P, CW, 2, 512], dt.bfloat16, tag="v")

                # ---- Q (feature-major): psum[f_tile, t] ----
                if c == 0:
                    # ec-outer so the first matmul only needs the first two
                    # small DMAs; uses all 4 psum accumulators live (fg3
                    # borrows the two 1-bank out-projection buffers).
                    ps_qs = [ps_alloc(i) for i in range(3)]
                    ps_qs.append([psF.tile([P, 512], dt.float32, tag="f",
                                           name=f"ps_q3{hh}")[:, :]
                                  for hh in range(2)])
                    for ec in range(EC):
                        for fg in range(4):
                            for half in range(2):
                                ft = fg * 2 + half
                                nc.tensor.matmul(
                                    ps_qs[fg][half][:, :tcv],
                                    wqQ[ec][:, ft * P:ft * P + P],
                                    xt_ec(ec, slice(0, tcv)),
                                    start=(ec == 0),
                                    stop=(ec == EC - 1),
                                )
                    for fg in range(4):
                        for half in range(2):
                            ft = fg * 2 + half
                            if has_bqk:
                                nc.scalar.activation(
                                    q_sb[:, ft, :tcv], ps_qs[fg][half][:, :tcv],
                                    AF.Identity, bias=bqk[:, 0, ft:ft + 1],
                                )
                            else:
                                nc.scalar.activation(
                                    q_sb[:, ft, :tcv], ps_qs[fg][half][:, :tcv],
                                    AF.Copy,
                                )
                else:
                    for fg in range(4):
                        ps = ps_alloc(fg)
                        for half in range(2):
                            ft = fg * 2 + half
                            for ec in range(EC):
                                nc.tensor.matmul(
                                    ps[half][:, :tcv],
                                    wqQ[ec][:, ft * P:ft * P + P],
                                    xt_ec(ec, slice(0, tcv)),
                                    start=(ec == 0),
                                    stop=(ec == EC - 1),
                                )
                        for half in range(2):
                            ft = fg * 2 + half
                            if has_bqk:
                                nc.scalar.activation(
                                    q_sb[:, ft, :tcv], ps[half][:, :tcv],
                                    AF.Identity, bias=bqk[:, 0, ft:ft + 1],
                                )
                            else:
                                nc.scalar.activation(
                                    q_sb[:, ft, :tcv], ps[half][:, :tcv],
                                    AF.Copy,
                                )

                # ---- K (feature-major). Evicted into k_tiles: head 2*ft+hh
                # lives at partitions 64hh.. of tile fg, half ft%2. Padded key
                # columns (last chunk) are memset to zero: the reference's
                # zero-padded x gives k=0 there, so padded keys contribute
                # exp(0)=1 with v=0. ----
                k_tiles = [kp.tile([P, 2, CT], dt.bfloat16, tag=f"k{fg}",
                                   name=f"k{fg}")
                           for fg in range(4)]
                for fg in range(4):
                    ps = ps_alloc(fg)
                    for half in range(2):
                        ft = fg * 2 + half
                        off = ft * P
                        for ec in range(EC):
                            nc.tensor.matmul(
                                ps[half][:, :tcv],
                                wq[:, ec, off:off + P],
                                xt_ec(ec, slice(0, tcv)),
                                start=(ec == 0),
                                stop=(ec == EC - 1),
                            )
                    if tcv < CT:
                        nc.gpsimd.memset(k_tiles[fg][:, :, tcv:], 0.0)
                    for half in range(2):
                        ft = fg * 2 + half
                        if has_bqk:
                            nc.scalar.activation(
                                k_tiles[fg][:, half, :tcv], ps[half][:, :tcv],
                                AF.Identity, bias=bqk[:, 1, ft:ft + 1],
                            )
                        else:
                            nc.scalar.activation(
                                k_tiles[fg][:, half, :tcv], ps[half][:, :tcv],
                                AF.Copy,
                            )

                # ---- V (token-major): psum[t, f] per window ----
                for wi in range(CW):
                    ps = ps_alloc(wi)
                    for fh in range(2):
                        off = E + fh * 512
                        for ec in range(EC):
                            nc.tensor.matmul(
                                ps[fh][:, :],
                                xt_ec(ec, slice(wi * W, (wi + 1) * W)),
                                wq[:, ec, off:off + 512],
                                start=(ec == 0),
                                stop=(ec == EC - 1),
                            )
                    for fh in range(2):
                        nc.vector.tensor_copy(v_sb[:, wi, fh], ps[fh][:, :])

                # ---- attention (A) + out-projection (B), software-pipelined:
                # B(w-1) is emitted inside A(w) so the PE has big streams to
                # hide the D-chain ldweights and the evict->normalize latency.
                # Window body; the out-projection runs with a lag of TWO
                # windows (bmm(w-2) inside window w) so its o stationaries
                # are always long-ready and the PE never waits on the
                # r-evict/o-multiply chain.
                for wi in range(CW):
                    g = c * CW + wi
                    row0 = g * W
                    rows = min(s_out - row0, W)
                    if rows <= 0:
                        continue
                    tq = rows
                    eslot, (ps_s0, ps_s1) = stage_a1(wi, k_tiles, q_sb, tq)
                    # early-pop in window 1: the lag-2 pipeline leaves the
                    # first windows without out-projection fill, and the bare
                    # dependency-chain idle can trip the HAM clock-gate into
                    # a mid-kernel re-throttle.
                    pend = pends.pop(0) if (len(pends) >= 2 or
                                            (g <= 2 and pends)) else None
                    if pend is not None:
                        ps_f0 = psF.tile([P, 512], dt.float32, tag="f",
                                         name="ps_f0")
                        for cc in range(EC):
                            bmm_one(ps_f0, pend[0], 0, cc)
                    rd16 = stage_d16(eslot, ps_s0, tq)
                    o_un = stage_av(wi, eslot, ps_s1, v_sb, tq)
                    ps_rs = stage_r(rd16, tq)
                    o_halves = stage_muls(o_un, ps_rs, tq)
                    if pend is not None:
                        stage_b_out_half(ps_f0, pend[1], pend[2], 0)
                        ps_f1 = psF.tile([P, 512], dt.float32, tag="f",
                                         name="ps_f1")
                        for cc in range(EC):
                            bmm_one(ps_f1, pend[0], 1, cc)
                        stage_b_out_half(ps_f1, pend[1], pend[2], 1)
                    pends.append((o_halves, row0, rows))

            for pend in pends:
                ps_f0 = psF.tile([P, 512], dt.float32, tag="f", name="ps_f0")
                for cc in range(EC):
                    bmm_one(ps_f0, pend[0], 0, cc)
                stage_b_out_half(ps_f0, pend[1], pend[2], 0)
                ps_f1 = psF.tile([P, 512], dt.float32, tag="f", name="ps_f1")
                for cc in range(EC):
                    bmm_one(ps_f1, pend[0], 1, cc)
                stage_b_out_half(ps_f1, pend[1], pend[2], 1)

    nc.compile()
    return nc


def prep_inputs(x, w_qkv, b_qkv, w_out, b_out, n_chunks, s_out):
    """Host-side staging: pad, transpose, cast, fold scale into w_q."""
    sp = n_chunks * CT
    nb = x.shape[0]

    wqkvT = np.ascontiguousarray(w_qkv.T).astype(np.float32).copy()
    wqkvT[:, :E] *= 1.0 / np.sqrt(D)
    wqkv_sb = np.ascontiguousarray(
        wqkvT.reshape(EC, P, 3 * E).transpose(1, 0, 2)
    ).astype(BF16)

    woutT = np.ascontiguousarray(w_out.T)
    wout_sb = np.ascontiguousarray(
        woutT.reshape(EC, P, E).transpose(1, 0, 2)
    ).astype(BF16)

    oh = np.zeros((P, H, H), dtype=BF16)
    for h in range(H):
        oh[:, h, h] = 1

    # rd16 row r holds 1/den of e-slot r = head (r//8)*8 + POS[r%8];
    # output partition p of chunk cc wants head 2cc + p//64.
    selm = np.zeros((H, EC, P), dtype=F16)
    for cc in range(EC):
        for m in range(P):
            h = 2 * cc + m // D
            r = (h // 8) * 8 + SLOT[h % 8]
            selm[r, cc, m] = 1

    base = {"wqkv": wqkv_sb, "wout": wout_sb, "onehot": oh, "sel": selm}

    has_bqk = bool(np.any(b_qkv[:2 * E]))
    has_bout = bool(np.any(b_out)) or bool(np.any(b_qkv[2 * E:]))
    if has_bqk:
        bqk = np.stack(
            [b_qkv[:E].reshape(EC, P).T / np.sqrt(D),
             b_qkv[E:2 * E].reshape(EC, P).T], axis=1
        ).astype(np.float32)  # (P, 2, EC)
        base["bqk"] = np.ascontiguousarray(bqk)
    if has_bout:
        cbv = (b_out + b_qkv[2 * E:] @ w_out.T).astype(np.float32)  # (E,)
        base["cb"] = np.ascontiguousarray(
            np.broadcast_to(cbv.reshape(1, 2, 512), (P, 2, 512))
        ).copy()

    in_maps = []
    for b in range(nb):
        xp_ = np.zeros((sp, E), dtype=np.float32)
        xp_[:min(s_out, x.shape[1])] = x[b][:s_out]
        xT = np.ascontiguousarray(xp_.T)  # (E, sp)
        xt_sb = np.ascontiguousarray(
            xT.reshape(EC, P, n_chunks, CT).transpose(2, 1, 0, 3)
        ).astype(BF16)  # (n_chunks, P, EC, CT)
        m = dict(base)
        m["xt"] = xt_sb
        in_maps.append(m)
    return in_maps, has_bqk, has_bout


def run(x, w_qkv, b_qkv, w_out, b_out, n_chunks=NW // CW, s_out=S, trace=False):
    from concourse import bass_utils

    in_maps, has_bqk, has_bout = prep_inputs(
        x, w_qkv, b_qkv, w_out, b_out, n_chunks, s_out
    )
    key = (n_chunks, s_out, has_bqk, has_bout)
    if key not in _cache:
        _cache[key] = build_nc(*key)
    nc = _cache[key]

    res = bass_utils.run_bass_kernel_spmd(
        nc, in_maps, core_ids=list(range(len(in_maps))), trace=trace,
    )
    out = np.stack([r["out"] for r in res.results], axis=0)
    return out, res


def kernel(x, w_qkv, b_qkv, w_out, b_out):
    x = np.asarray(x, dtype=np.float32)
    w_qkv = np.asarray(w_qkv, dtype=np.float32)
    b_qkv = np.asarray(b_qkv, dtype=np.float32)
    w_out = np.asarray(w_out, dtype=np.float32)
    b_out = np.asarray(b_out, dtype=np.float32)
    out, _ = run(x, w_qkv, b_qkv, w_out, b_out)
    return out
